# revision 52
# baseline (speedup 1.0000x reference)
"""Trainium2 Bass kernel for nn_DConv (shift-gather + 3x3 conv), 8 NeuronCores.

Math: the reference's per-channel torch.roll on the zero-padded image only
wraps into zero-pad rows/columns, so the op collapses to a 3x3 conv over a
host-pre-shifted, zero-padded image (the roll + pad are pure data layout,
applied while packing partitions on the host).

PE packing (the key trick): instead of batching two samples into one
block-masked matmul (50% useful PE occupancy), pack TWO IMAGE ROWS into the
contraction dim and TWO OUTPUT ROWS into the output dim:

  K = 128 = 64 ci x {even row, odd row}   (partition p<64: ci=p, padded rows
      2j at slot j; p>=64: ci=p-64, padded rows 2j+1 at slot j)
  M = 128 = 64 co x {even out row, odd out row}

An output row pair (h, h+1), h even, needs padded rows h..h+3 = slots
h/2, h/2+1.  Six accumulating matmuls (2 slots x 3 kw taps) with fixed
128x128 block weights (3 of 4 blocks nonzero = 75% useful density) produce
both output rows for all 64 out-channels.  Per core (2 samples, processed
back to back) the PE streams 2 x 80 x 6 x 160 = 153,600 moving columns
vs 230,400 for the sample-pair scheme -- a 1.5x matmul-time cut.

Samples are data-parallel over batch: 2 per core, computed sequentially on
the full 128-partition array.

HBM output layout is (sample, row-parity, co, h/2, w) so each partition's
store rows are contiguous in HBM (multi-row >=512B descriptors at full DMA
rate); the host de-interleaves parity when unpacking (free).

Backpressure control: each sample gets a full-size SBUF staging buffer, so
PSUM->SBUF copies never wait on store DMAs (store transfers queue behind
the big input-band loads on the shared DMA engines; with rotating staging
that dependency chain stalled the PE mid-kernel and reset the p-state).

Dtype: bfloat16 operands, fp32 PSUM accumulation; output stored as bf16
and upcast on the host.

Tail: sample 1 computes pairs 72-79 FIRST (their slots are in its
first-loaded band) so those stores happen mid-kernel; the remaining
chunks taper to single pairs so their PSUM->SBUF copies stay off the DVE
when the final copy needs it, and the last two computed pairs (71, 74)
go to a two-slot bf16 sidecar (host merges) so the final bf16 batch's
descriptor gen clears the shared HWDGE before the sidecar's gen request.
Each end-of-kernel DMA is alone on its engine ring: the sequencers are
in-order, so a DMA parked on a not-yet-fired dependency would
head-of-line-block anything behind it.

Schedule: PE ramps on dummy matmuls over a zeroed tile from ~t=0; the
weights ride at the head of the x tensor so ONE first DMA delivers the
tap matrices plus the first image slots (a single gen+DGE+transfer+sem
latency, ~3.8us, instead of two serialized ones); sample 0 then uses a
soft-start chunk plan sized to the remaining band-load arrivals; loads
are large contiguous row-band DMAs alternating the two HWDGE rings;
mid-kernel stores run on SWDGE (gpsimd) so they can't head-of-line-block
the loads.
"""
import numpy as np
import ml_dtypes

from concourse import bacc, tile, mybir
from concourse.bass_utils import run_bass_kernel_spmd

# problem shape (hardcoded per contract)
B, C, H, W = 16, 64, 160, 160
N_CORES = 8
B_PER_CORE = B // N_CORES  # 2
VP = W + 2                 # padded col pitch 162
SLOTS = (H + 2) // 2       # 81 row-pair slots (padded rows 0..161)

BF16 = mybir.dt.bfloat16
F32 = mybir.dt.float32

# shift table: group g = ci % 5
DXS = [0, 1, 0, -1, 0]
DYS = [0, 0, 1, 0, -1]

# chunk plans: (pair_base, n_pairs) per PSUM chunk (n_pairs*160 <= 512).
# sample 0 soft-starts so compute begins on the first loaded slots; sample 1
# tapers to 2-pair chunks for a short store tail, with the last chunks
# going to the sidecar.
S0_CHUNKS = [(0, 1), (1, 1), (2, 1), (3, 1), (4, 2)] + \
    [(6 + 3 * i, 3) for i in range(24)] + [(78, 2)]
# sample 1 computes pairs 72-73 and 75-79 FIRST (their slots are in the
# first-loaded band) so their stores all happen mid-kernel; the single
# remaining tail chunks taper to single pairs, with pairs 71 and 74 in
# a small bf16 sidecar that is then the ONLY DMA on the
# end-of-kernel critical path -- no HWDGE-gen queueing, no DMA-engine
# queueing, no store fan-out after the final matmul
S1_CHUNKS = [(72, 2), (75, 3), (78, 2)] + \
    [(3 * i, 3) for i in range(23)] + \
    [(69, 1), (70, 1), (71, 1), (74, 1)]
# store batches (b0, b1, engine): flushed when the chunk ending at b1
# completes.  Mid-kernel batches ride SWDGE (gpsimd) so they can't
# head-of-line-block the HWDGE load rings; the last-flushed batch gets
# the scalar ring, and the sidecar the sync ring, so each late DMA is
# alone on its queue (engine SEQs are in-order: a parked DMA blocks
# everything behind it on that ring)
S0_BATCHES = [(0, 18, "gpsimd"), (18, 36, "gpsimd"), (36, 54, "gpsimd"),
              (54, 72, "gpsimd"), (72, 80, "gpsimd")]
S1_BATCHES = [(72, 74, "gpsimd"), (75, 78, "gpsimd"), (78, 80, "gpsimd"),
              (0, 18, "gpsimd"), (18, 36, "gpsimd"), (36, 54, "gpsimd"),
              (54, 63, "gpsimd"), (63, 69, "gpsimd"), (69, 71, "scalar")]
# row-band loads (slot ranges, engine) per sample; sample 0's first band
# is fused with the weight load (one DMA = weights + slots 0-2, so the
# first chunk's operands arrive in a single DMA latency); sample 1's top
# band goes first (its pairs 72-79 run first)
S0_BANDS = [(4, 8, "scalar"), (8, 16, "sync"), (16, 36, "scalar"),
            (36, 58, "sync"), (58, 81, "scalar")]
S1_BANDS = [(54, 81, "sync"), (0, 27, "scalar"), (27, 54, "sync")]

# the weights split: pass-A taps (0-2) ride the head DMA with slots 0-3
# (shortest possible first-DMA transfer); pass-B taps (3-5) follow in a
# parallel tiny DMA that lands before the first chunk needs tap 3
WTA_ELEMS = 3 * 128                 # 384 bf16 pass-A weight elems
WTB_ELEMS = 3 * 128                 # 384 bf16 pass-B weight elems
S_ELEMS = SLOTS * VP                # 13122 image elems per sample
# flat per-partition x layout: [wtA | sample0 | wtB | sample1]
X_FLAT = WTA_ELEMS + WTB_ELEMS + 2 * S_ELEMS

WARMUP_MMS = 12            # dummy 256-col matmuls ramping the PE from ~t=0


def build_kernel(reps: int = 1, timing: bool = False):
    nc = bacc.Bacc("TRN2", target_bir_lowering=False, debug=False,
                   num_devices=N_CORES)
    if timing:
        # timing-only variant: big tensors stay in device DRAM (uninitialised
        # garbage is fine for timing) so per-call host<->device transfer is
        # tiny and wall-clock noise is dominated by the fixed RTT only.
        x_dram = nc.dram_tensor("x", [128, X_FLAT], BF16)
        out_dram = nc.dram_tensor(
            "out", [B_PER_CORE, 2, C, H // 2, W], BF16)
        tail_dram = nc.dram_tensor("out_tail", [128, 2, W], BF16)
        dummy = nc.dram_tensor("t_dummy", [1, 16], BF16,
                               kind="ExternalOutput")
    else:
        x_dram = nc.dram_tensor("x", [128, X_FLAT], BF16,
                                kind="ExternalInput")
        out_dram = nc.dram_tensor(
            "out", [B_PER_CORE, 2, C, H // 2, W], BF16,
            kind="ExternalOutput")
        tail_dram = nc.dram_tensor("out_tail", [128, 2, W], BF16,
                                   kind="ExternalOutput")
    x_ap = x_dram.ap()
    # per-sample slot views of the flat layout [wtA | sample0 | wtB | sample1]
    _s_off = [WTA_ELEMS, WTA_ELEMS + S_ELEMS + WTB_ELEMS]
    x_src = [
        x_ap[:, _s_off[s]:_s_off[s] + S_ELEMS]
        .rearrange("p (j v) -> p j v", v=VP)
        for s in range(B_PER_CORE)
    ]
    # partition view: p = parity*64 + co
    out_view = out_dram.ap().rearrange("s par co hp w -> s (par co) hp w")

    with tile.TileContext(nc) as tc:
        with (
            tc.tile_pool(name="wpool", bufs=1) as wpool,
            tc.tile_pool(name="psum", bufs=6, space="PSUM") as psum_pool,
        ):
            # tiny zero tile for PE warmup: dummy matmuls start at ~t=0 with
            # no input dependency, burning the PE p-state ramp while the
            # weights and first image rows arrive
            wz = wpool.tile([128, 256], BF16, tag="wz")
            nc.gpsimd.memset(wz[:].bitcast(F32), 0.0)

            # [pass-A weights | sample-0 image] share one tile so the first
            # DMA delivers weights AND the first slots in a single transfer;
            # pass-B weights land in a parallel tiny DMA just in time for
            # the first chunk's 4th matmul
            comb = wpool.tile([128, WTA_ELEMS + S_ELEMS], BF16, tag="comb")
            wta = comb[:, 0:WTA_ELEMS].rearrange("p (t m) -> p t m", m=128)
            xs0 = comb[:, WTA_ELEMS:].rearrange("p (j v) -> p j v", v=VP)
            wtb_t = wpool.tile([128, WTB_ELEMS], BF16, tag="wtb")
            wtb = wtb_t[:].rearrange("p (t m) -> p t m", m=128)
            tailbuf = wpool.tile([128, 2, W], BF16, tag="tail")
            xs1 = wpool.tile([128, SLOTS, VP], BF16, tag="xs1")
            xs = [xs0, xs1]
            # full-size staging per sample: stores never backpressure copies
            stg0 = wpool.tile([128, 80, W], BF16, tag="stg0")
            stg1 = wpool.tile([128, 80, W], BF16, tag="stg1")
            stg = [stg0, stg1]

            if WARMUP_MMS:
                psw = psum_pool.tile([128, 512], F32, tag="ps")
                for i in range(WARMUP_MMS):
                    nc.tensor.matmul(psw[:, 0:256], wz[:, 0:128], wz[:],
                                     start=(i == 0),
                                     stop=(i == WARMUP_MMS - 1))

            for _ in range(reps):
                # first DMA: pass-A weights + sample-0 slots 0-3
                head = WTA_ELEMS + 4 * VP
                nc.sync.dma_start(comb[:, 0:head], x_ap[:, 0:head])
                # pass-B weights, parallel on the same ring
                wtb_off = WTA_ELEMS + S_ELEMS
                nc.sync.dma_start(wtb_t[:],
                                  x_ap[:, wtb_off:wtb_off + WTB_ELEMS])
                # remaining row-band loads
                for s, bands in ((0, S0_BANDS), (1, S1_BANDS)):
                    for a, b, ename in bands:
                        getattr(nc, ename).dma_start(
                            xs[s][:, a:b, :], x_src[s][:, a:b, :])

                for s, chunks, batches in (
                        (0, S0_CHUNKS, S0_BATCHES),
                        (1, S1_CHUNKS, S1_BATCHES)):
                    bi = 0       # current batch index
                    last = len(chunks) - 1
                    for ci_, (jp, npair) in enumerate(chunks):
                        n_out = W * npair
                        ps = psum_pool.tile([128, 512], F32, tag="ps")
                        ps_view = ps[:, 0:n_out].rearrange(
                            "p (r v) -> p r v", v=W)
                        for t in range(6):
                            pas, kw = divmod(t, 3)
                            lhsT = (wta[:, t, :] if t < 3
                                    else wtb[:, t - 3, :])
                            nc.tensor.matmul(
                                ps_view[:],
                                lhsT,
                                xs[s][:, jp + pas:jp + pas + npair,
                                      kw:kw + W],
                                start=(t == 0),
                                stop=(t == 5),
                            )
                        if s == 1 and ci_ >= last - 1:
                            # final two computed pairs (71 and 74) both go
                            # to the bf16 sidecar (host merges), so the last
                            # bf16 batch's descriptor gen clears the HWDGE
                            # before the sidecar's gen request -- exactly
                            # one DMA on the end-of-kernel critical path
                            nc.vector.tensor_copy(
                                tailbuf[:, ci_ - (last - 1), :],
                                ps[:, 0:n_out])
                            if ci_ == last:
                                nc.sync.dma_start(tail_dram.ap()[:],
                                                  tailbuf[:])
                            continue
                        nc.vector.tensor_copy(
                            stg[s][:, jp:jp + npair, :], ps_view[:])
                        if bi < len(batches) and jp + npair == batches[bi][1]:
                            # flush the batch: rows are hp-contiguous in HBM
                            # so each partition is one big descriptor
                            b0, b1, ename = batches[bi]
                            getattr(nc, ename).dma_start(
                                out_view[s, :, b0:b1, :],
                                stg[s][:, b0:b1, :],
                            )
                            bi += 1
            if timing:
                nc.sync.dma_start(dummy.ap()[:], wta[0:1, 0, 0:16])
    nc.compile()
    return nc


def _host_inputs(x: np.ndarray, weight: np.ndarray):
    """Pack the shifted + padded per-channel images into the row-interleaved
    partition layout (bf16) with the 6 block tap matrices at the head of
    each partition: flat per-partition layout [wt | sample0 | sample1]."""
    xv = np.asarray(x, dtype=np.float32).reshape(
        N_CORES, B_PER_CORE, C, H, W)
    xp = np.zeros((N_CORES, B_PER_CORE, C, H + 2, W + 2), np.float32)
    xp[:, :, :, 1:H + 1, 1:W + 1] = xv
    for g in range(5):
        ch = (np.arange(C) % 5) == g
        xp[:, :, ch] = np.roll(xp[:, :, ch], (DYS[g], DXS[g]), axis=(3, 4))
    # [cores, 128, samples, slots, cols]: p<64 even rows, p>=64 odd rows
    full = np.empty((N_CORES, 128, B_PER_CORE, SLOTS, VP), np.float32)
    full[:, 0:64] = xp[:, :, :, 0::2, :].transpose(0, 2, 1, 3, 4)
    full[:, 64:128] = xp[:, :, :, 1::2, :].transpose(0, 2, 1, 3, 4)

    # block tap matrices [128(k), 6, 128(m)]; k = (row parity, ci),
    # m = (out-row parity, co); t = pass*3 + kw
    wk = np.asarray(weight, dtype=np.float32).transpose(1, 2, 3, 0)
    # wk[ci, kh, kw, co]
    wt_host = np.zeros((128, 6, 128), np.float32)
    for kw in range(3):
        # pass A: slot h/2 = padded rows (h, h+1)
        wt_host[0:64, kw, 0:64] = wk[:, 0, kw, :]        # row h   -> out h
        wt_host[64:128, kw, 0:64] = wk[:, 1, kw, :]      # row h+1 -> out h
        wt_host[64:128, kw, 64:128] = wk[:, 0, kw, :]    # row h+1 -> out h+1
        # pass B: slot h/2+1 = padded rows (h+2, h+3)
        wt_host[0:64, 3 + kw, 0:64] = wk[:, 2, kw, :]    # row h+2 -> out h
        wt_host[0:64, 3 + kw, 64:128] = wk[:, 1, kw, :]  # row h+2 -> out h+1
        wt_host[64:128, 3 + kw, 64:128] = wk[:, 2, kw, :]  # row h+3 -> h+1

    x_flat = np.empty((N_CORES, 128, X_FLAT), np.float32)
    # flat layout [wtA | sample0 | wtB | sample1]
    fl = full.reshape(N_CORES, 128, 2, S_ELEMS)
    x_flat[:, :, 0:WTA_ELEMS] = wt_host[:, 0:3, :].reshape(128, WTA_ELEMS)
    x_flat[:, :, WTA_ELEMS:WTA_ELEMS + S_ELEMS] = fl[:, :, 0]
    wb0 = WTA_ELEMS + S_ELEMS
    x_flat[:, :, wb0:wb0 + WTB_ELEMS] = \
        wt_host[:, 3:6, :].reshape(128, WTB_ELEMS)
    x_flat[:, :, wb0 + WTB_ELEMS:] = fl[:, :, 1]
    return x_flat.astype(ml_dtypes.bfloat16)


_NC_CACHE = {}


def _get_nc(reps: int = 1):
    if reps not in _NC_CACHE:
        _NC_CACHE[reps] = build_kernel(reps)
    return _NC_CACHE[reps]


def kernel(x: np.ndarray, weight: np.ndarray) -> np.ndarray:
    x = np.asarray(x, dtype=np.float32)
    weight = np.asarray(weight, dtype=np.float32)
    x_packed = _host_inputs(x, weight)
    nc = _get_nc(1)
    in_maps = [
        {"x": np.ascontiguousarray(x_packed[k])}
        for k in range(N_CORES)
    ]
    try:
        res = run_bass_kernel_spmd(nc, in_maps,
                                   core_ids=list(range(N_CORES)))
    except Exception:
        # transient device errors (e.g. NRT_EXEC_UNIT_UNRECOVERABLE) have
        # been observed on this fabric; one retry is cheap insurance
        import time
        time.sleep(5)
        res = run_bass_kernel_spmd(nc, in_maps,
                                   core_ids=list(range(N_CORES)))
    out = np.empty((B, C, H, W), np.float32)
    for k in range(N_CORES):
        r = np.asarray(res.results[k]["out"]).astype(np.float32)
        # r[s, par, co, hp, w] -> out rows 2*hp + par
        for s in range(B_PER_CORE):
            ov = out[k * B_PER_CORE + s].reshape(C, H // 2, 2, W)
            ov[:, :, 0, :] = r[s, 0]
            ov[:, :, 1, :] = r[s, 1]
        # sample 1 rows 142/143 (pair 71) and 148/149 (pair 74) come from
        # the bf16 sidecar
        tail = np.asarray(res.results[k]["out_tail"]).astype(np.float32)
        o1 = out[k * B_PER_CORE + 1]
        o1[:, 142, :] = tail[0:64, 0]
        o1[:, 143, :] = tail[64:128, 0]
        o1[:, 148, :] = tail[0:64, 1]
        o1[:, 149, :] = tail[64:128, 1]
    return out


# revision 53
# speedup vs baseline: 1.0240x; 1.0240x over previous
"""Trainium2 Bass kernel for nn_DConv (shift-gather + 3x3 conv), 8 NeuronCores.

Math: the reference's per-channel torch.roll on the zero-padded image only
wraps into zero-pad rows/columns, so the op collapses to a 3x3 conv over a
host-pre-shifted, zero-padded image (the roll + pad are pure data layout,
applied while packing partitions on the host).

PE packing (the key trick): instead of batching two samples into one
block-masked matmul (50% useful PE occupancy), pack TWO IMAGE ROWS into the
contraction dim and TWO OUTPUT ROWS into the output dim:

  K = 128 = 64 ci x {even row, odd row}   (partition p<64: ci=p, padded rows
      2j at slot j; p>=64: ci=p-64, padded rows 2j+1 at slot j)
  M = 128 = 64 co x {even out row, odd out row}

An output row pair (h, h+1), h even, needs padded rows h..h+3 = slots
h/2, h/2+1.  Six accumulating matmuls (2 slots x 3 kw taps) with fixed
128x128 block weights (3 of 4 blocks nonzero = 75% useful density) produce
both output rows for all 64 out-channels.  Per core (2 samples, processed
back to back) the PE streams 2 x 80 x 6 x 160 = 153,600 moving columns
vs 230,400 for the sample-pair scheme -- a 1.5x matmul-time cut.

Samples are data-parallel over batch: 2 per core, computed sequentially on
the full 128-partition array.

HBM output layout is (sample, row-parity, co, h/2, w) so each partition's
store rows are contiguous in HBM (multi-row >=512B descriptors at full DMA
rate); the host de-interleaves parity when unpacking (free).

Backpressure control: each sample gets a full-size SBUF staging buffer, so
PSUM->SBUF copies never wait on store DMAs (store transfers queue behind
the big input-band loads on the shared DMA engines; with rotating staging
that dependency chain stalled the PE mid-kernel and reset the p-state).

Dtype: bfloat16 operands, fp32 PSUM accumulation; output stored as bf16
and upcast on the host.

Tail: sample 1 computes pairs 72-79 FIRST (their slots are in its
first-loaded band) so those stores happen mid-kernel; the remaining
chunks taper to single pairs so their PSUM->SBUF copies stay off the DVE
when the final copy needs it, and the last two computed pairs (71, 74)
go to a two-slot bf16 sidecar (host merges) so the final bf16 batch's
descriptor gen clears the shared HWDGE before the sidecar's gen request.
Each end-of-kernel DMA is alone on its engine ring: the sequencers are
in-order, so a DMA parked on a not-yet-fired dependency would
head-of-line-block anything behind it.

Schedule: PE ramps on dummy matmuls over a zeroed tile from ~t=0; the
weights ride at the head of the x tensor so ONE first DMA delivers the
tap matrices plus the first image slots (a single gen+DGE+transfer+sem
latency, ~3.8us, instead of two serialized ones); sample 0 then uses a
soft-start chunk plan sized to the remaining band-load arrivals; loads
are large contiguous row-band DMAs alternating the two HWDGE rings;
mid-kernel stores run on SWDGE (gpsimd) so they can't head-of-line-block
the loads.
"""
import numpy as np
import ml_dtypes

from concourse import bacc, tile, mybir
from concourse.bass_utils import run_bass_kernel_spmd

# problem shape (hardcoded per contract)
B, C, H, W = 16, 64, 160, 160
N_CORES = 8
B_PER_CORE = B // N_CORES  # 2
VP = W + 2                 # padded col pitch 162
SLOTS = (H + 2) // 2       # 81 row-pair slots (padded rows 0..161)

BF16 = mybir.dt.bfloat16
F32 = mybir.dt.float32

# shift table: group g = ci % 5
DXS = [0, 1, 0, -1, 0]
DYS = [0, 0, 1, 0, -1]

# chunk plans: (pair_base, n_pairs) per PSUM chunk (n_pairs*160 <= 512).
# sample 0 soft-starts so compute begins on the first loaded slots; sample 1
# tapers to 2-pair chunks for a short store tail, with the last chunks
# going to the sidecar.
S0_CHUNKS = [(0, 1), (1, 1), (2, 2), (4, 2)] + \
    [(6 + 3 * i, 3) for i in range(24)] + [(78, 2)]
# sample 1 computes pairs 72-73 and 75-79 FIRST (their slots are in the
# first-loaded band) so their stores all happen mid-kernel; the single
# remaining tail chunks taper to single pairs, with pairs 71 and 74 in
# a small bf16 sidecar that is then the ONLY DMA on the
# end-of-kernel critical path -- no HWDGE-gen queueing, no DMA-engine
# queueing, no store fan-out after the final matmul
S1_CHUNKS = [(72, 2), (75, 3), (78, 2)] + \
    [(3 * i, 3) for i in range(23)] + \
    [(69, 1), (70, 1), (71, 1), (74, 1)]
# store batches (b0, b1, engine): flushed when the chunk ending at b1
# completes.  Mid-kernel batches ride SWDGE (gpsimd) so they can't
# head-of-line-block the HWDGE load rings; the last-flushed batch gets
# the scalar ring, and the sidecar the sync ring, so each late DMA is
# alone on its queue (engine SEQs are in-order: a parked DMA blocks
# everything behind it on that ring)
S0_BATCHES = [(0, 18, "gpsimd"), (18, 36, "gpsimd"), (36, 54, "gpsimd"),
              (54, 72, "gpsimd"), (72, 80, "gpsimd")]
S1_BATCHES = [(72, 74, "gpsimd"), (75, 78, "gpsimd"), (78, 80, "gpsimd"),
              (0, 18, "gpsimd"), (18, 36, "gpsimd"), (36, 54, "gpsimd"),
              (54, 63, "gpsimd"), (63, 69, "gpsimd"), (69, 71, "scalar")]
# row-band loads (slot ranges, engine) per sample; sample 0's first band
# is fused with the weight load (one DMA = weights + slots 0-2, so the
# first chunk's operands arrive in a single DMA latency); sample 1's top
# band goes first (its pairs 72-79 run first)
S0_BANDS = [(3, 10, "scalar"), (10, 25, "sync"), (25, 53, "scalar"),
            (53, 81, "sync")]
S1_BANDS = [(54, 81, "scalar"), (0, 27, "sync"), (27, 54, "scalar")]

WT_ELEMS = 6 * 128                  # 768 bf16 weight elems per partition
S_ELEMS = SLOTS * VP                # 13122 image elems per sample
X_FLAT = WT_ELEMS + 2 * S_ELEMS    # flat per-partition x layout

WARMUP_MMS = 13            # dummy 256-col matmuls ramping the PE from ~t=0


def build_kernel(reps: int = 1, timing: bool = False):
    nc = bacc.Bacc("TRN2", target_bir_lowering=False, debug=False,
                   num_devices=N_CORES)
    if timing:
        # timing-only variant: big tensors stay in device DRAM (uninitialised
        # garbage is fine for timing) so per-call host<->device transfer is
        # tiny and wall-clock noise is dominated by the fixed RTT only.
        x_dram = nc.dram_tensor("x", [128, X_FLAT], BF16)
        out_dram = nc.dram_tensor(
            "out", [B_PER_CORE, 2, C, H // 2, W], BF16)
        tail_dram = nc.dram_tensor("out_tail", [128, 2, W], BF16)
        dummy = nc.dram_tensor("t_dummy", [1, 16], BF16,
                               kind="ExternalOutput")
    else:
        x_dram = nc.dram_tensor("x", [128, X_FLAT], BF16,
                                kind="ExternalInput")
        out_dram = nc.dram_tensor(
            "out", [B_PER_CORE, 2, C, H // 2, W], BF16,
            kind="ExternalOutput")
        tail_dram = nc.dram_tensor("out_tail", [128, 2, W], BF16,
                                   kind="ExternalOutput")
    x_ap = x_dram.ap()
    # per-sample slot views of the flat x layout [wt | sample0 | sample1]
    x_src = [
        x_ap[:, WT_ELEMS + s * S_ELEMS:WT_ELEMS + (s + 1) * S_ELEMS]
        .rearrange("p (j v) -> p j v", v=VP)
        for s in range(B_PER_CORE)
    ]
    # partition view: p = parity*64 + co
    out_view = out_dram.ap().rearrange("s par co hp w -> s (par co) hp w")

    with tile.TileContext(nc) as tc:
        with (
            tc.tile_pool(name="wpool", bufs=1) as wpool,
            tc.tile_pool(name="psum", bufs=6, space="PSUM") as psum_pool,
        ):
            # tiny zero tile for PE warmup: dummy matmuls start at ~t=0 with
            # no input dependency, burning the PE p-state ramp while the
            # weights and first image rows arrive
            wz = wpool.tile([128, 256], BF16, tag="wz")
            nc.gpsimd.memset(wz[:].bitcast(F32), 0.0)

            # [weights | sample-0 image] share one tile so the first DMA
            # delivers the weights AND the first slots in a single transfer
            comb = wpool.tile([128, WT_ELEMS + S_ELEMS], BF16, tag="comb")
            wt = comb[:, 0:WT_ELEMS].rearrange("p (t m) -> p t m", m=128)
            xs0 = comb[:, WT_ELEMS:].rearrange("p (j v) -> p j v", v=VP)
            tailbuf = wpool.tile([128, 2, W], BF16, tag="tail")
            xs1 = wpool.tile([128, SLOTS, VP], BF16, tag="xs1")
            xs = [xs0, xs1]
            # full-size staging per sample: stores never backpressure copies
            stg0 = wpool.tile([128, 80, W], BF16, tag="stg0")
            stg1 = wpool.tile([128, 80, W], BF16, tag="stg1")
            stg = [stg0, stg1]

            if WARMUP_MMS:
                psw = psum_pool.tile([128, 512], F32, tag="ps")
                for i in range(WARMUP_MMS):
                    nc.tensor.matmul(psw[:, 0:256], wz[:, 0:128], wz[:],
                                     start=(i == 0),
                                     stop=(i == WARMUP_MMS - 1))

            for _ in range(reps):
                # first DMA: weights + sample-0 slots 0-2, one transfer
                head = WT_ELEMS + 3 * VP
                nc.sync.dma_start(comb[:, 0:head], x_ap[:, 0:head])
                # remaining row-band loads
                for s, bands in ((0, S0_BANDS), (1, S1_BANDS)):
                    for a, b, ename in bands:
                        getattr(nc, ename).dma_start(
                            xs[s][:, a:b, :], x_src[s][:, a:b, :])

                for s, chunks, batches in (
                        (0, S0_CHUNKS, S0_BATCHES),
                        (1, S1_CHUNKS, S1_BATCHES)):
                    bi = 0       # current batch index
                    last = len(chunks) - 1
                    for ci_, (jp, npair) in enumerate(chunks):
                        n_out = W * npair
                        ps = psum_pool.tile([128, 512], F32, tag="ps")
                        ps_view = ps[:, 0:n_out].rearrange(
                            "p (r v) -> p r v", v=W)
                        for t in range(6):
                            pas, kw = divmod(t, 3)
                            nc.tensor.matmul(
                                ps_view[:],
                                wt[:, t, :],
                                xs[s][:, jp + pas:jp + pas + npair,
                                      kw:kw + W],
                                start=(t == 0),
                                stop=(t == 5),
                            )
                        if s == 1 and ci_ >= last - 1:
                            # final two computed pairs (71 and 74) both go
                            # to the bf16 sidecar (host merges), so the last
                            # bf16 batch's descriptor gen clears the HWDGE
                            # before the sidecar's gen request -- exactly
                            # one DMA on the end-of-kernel critical path
                            nc.vector.tensor_copy(
                                tailbuf[:, ci_ - (last - 1), :],
                                ps[:, 0:n_out])
                            if ci_ == last:
                                nc.sync.dma_start(tail_dram.ap()[:],
                                                  tailbuf[:])
                            continue
                        nc.vector.tensor_copy(
                            stg[s][:, jp:jp + npair, :], ps_view[:])
                        if bi < len(batches) and jp + npair == batches[bi][1]:
                            # flush the batch: rows are hp-contiguous in HBM
                            # so each partition is one big descriptor
                            b0, b1, ename = batches[bi]
                            getattr(nc, ename).dma_start(
                                out_view[s, :, b0:b1, :],
                                stg[s][:, b0:b1, :],
                            )
                            bi += 1
            if timing:
                nc.sync.dma_start(dummy.ap()[:], wt[0:1, 0, 0:16])
    nc.compile()
    return nc


def _host_inputs(x: np.ndarray, weight: np.ndarray):
    """Pack the shifted + padded per-channel images into the row-interleaved
    partition layout (bf16) with the 6 block tap matrices at the head of
    each partition: flat per-partition layout [wt | sample0 | sample1]."""
    xv = np.asarray(x, dtype=np.float32).reshape(
        N_CORES, B_PER_CORE, C, H, W)
    xp = np.zeros((N_CORES, B_PER_CORE, C, H + 2, W + 2), np.float32)
    xp[:, :, :, 1:H + 1, 1:W + 1] = xv
    for g in range(5):
        ch = (np.arange(C) % 5) == g
        xp[:, :, ch] = np.roll(xp[:, :, ch], (DYS[g], DXS[g]), axis=(3, 4))
    # [cores, 128, samples, slots, cols]: p<64 even rows, p>=64 odd rows
    full = np.empty((N_CORES, 128, B_PER_CORE, SLOTS, VP), np.float32)
    full[:, 0:64] = xp[:, :, :, 0::2, :].transpose(0, 2, 1, 3, 4)
    full[:, 64:128] = xp[:, :, :, 1::2, :].transpose(0, 2, 1, 3, 4)

    # block tap matrices [128(k), 6, 128(m)]; k = (row parity, ci),
    # m = (out-row parity, co); t = pass*3 + kw
    wk = np.asarray(weight, dtype=np.float32).transpose(1, 2, 3, 0)
    # wk[ci, kh, kw, co]
    wt_host = np.zeros((128, 6, 128), np.float32)
    for kw in range(3):
        # pass A: slot h/2 = padded rows (h, h+1)
        wt_host[0:64, kw, 0:64] = wk[:, 0, kw, :]        # row h   -> out h
        wt_host[64:128, kw, 0:64] = wk[:, 1, kw, :]      # row h+1 -> out h
        wt_host[64:128, kw, 64:128] = wk[:, 0, kw, :]    # row h+1 -> out h+1
        # pass B: slot h/2+1 = padded rows (h+2, h+3)
        wt_host[0:64, 3 + kw, 0:64] = wk[:, 2, kw, :]    # row h+2 -> out h
        wt_host[0:64, 3 + kw, 64:128] = wk[:, 1, kw, :]  # row h+2 -> out h+1
        wt_host[64:128, 3 + kw, 64:128] = wk[:, 2, kw, :]  # row h+3 -> h+1

    x_flat = np.empty((N_CORES, 128, X_FLAT), np.float32)
    x_flat[:, :, 0:WT_ELEMS] = wt_host.reshape(128, WT_ELEMS)
    x_flat[:, :, WT_ELEMS:] = full.reshape(N_CORES, 128, 2 * S_ELEMS)
    return x_flat.astype(ml_dtypes.bfloat16)


_NC_CACHE = {}


def _get_nc(reps: int = 1):
    if reps not in _NC_CACHE:
        _NC_CACHE[reps] = build_kernel(reps)
    return _NC_CACHE[reps]


def kernel(x: np.ndarray, weight: np.ndarray) -> np.ndarray:
    x = np.asarray(x, dtype=np.float32)
    weight = np.asarray(weight, dtype=np.float32)
    x_packed = _host_inputs(x, weight)
    nc = _get_nc(1)
    in_maps = [
        {"x": np.ascontiguousarray(x_packed[k])}
        for k in range(N_CORES)
    ]
    try:
        res = run_bass_kernel_spmd(nc, in_maps,
                                   core_ids=list(range(N_CORES)))
    except Exception:
        # transient device errors (e.g. NRT_EXEC_UNIT_UNRECOVERABLE) have
        # been observed on this fabric; one retry is cheap insurance
        import time
        time.sleep(5)
        res = run_bass_kernel_spmd(nc, in_maps,
                                   core_ids=list(range(N_CORES)))
    out = np.empty((B, C, H, W), np.float32)
    for k in range(N_CORES):
        r = np.asarray(res.results[k]["out"]).astype(np.float32)
        # r[s, par, co, hp, w] -> out rows 2*hp + par
        for s in range(B_PER_CORE):
            ov = out[k * B_PER_CORE + s].reshape(C, H // 2, 2, W)
            ov[:, :, 0, :] = r[s, 0]
            ov[:, :, 1, :] = r[s, 1]
        # sample 1 rows 142/143 (pair 71) and 148/149 (pair 74) come from
        # the bf16 sidecar
        tail = np.asarray(res.results[k]["out_tail"]).astype(np.float32)
        o1 = out[k * B_PER_CORE + 1]
        o1[:, 142, :] = tail[0:64, 0]
        o1[:, 143, :] = tail[64:128, 0]
        o1[:, 148, :] = tail[0:64, 1]
        o1[:, 149, :] = tail[64:128, 1]
    return out
